# revision 2
# baseline (speedup 1.0000x reference)
"""Trainium2 Bass kernel: LinearCausalAttention (RoPE + strictly-causal QK^T + scores@V).

Inputs (full): Q (2,8,2048,256) f32, K (2,8,2048,256) f32, V (2,1,2048,256) f32.
Returns (out, scores): out (2,8,2048,256), scores (2,8,2048,2048), both f32.

Sharding: 16 (b,h) pairs over 8 cores, 2 pairs/core; both pairs on a core share b,
so V is loaded once per core.

Per (b,h) on-device pipeline:
  - RoPE applied in natural layout with *deinterleaved* feature output
    (even-roped feats -> cols 0:128, odd-roped -> 128:256). A feature
    permutation shared by Q and K leaves Q.K^T and scores@V unchanged.
  - PE transposes Qr/Kr to [n, t] layout for the QK^T matmuls (fp32r).
  - scores: per 128-row t-block, matmul s-chunks (<=512 wide), mask the
    diagonal 128 block with a strictly-lower mask, DMA only the causal
    columns (output buffers are pre-zeroed by the runtime contract).
  - out via linear-attention prefix state: A[n,d] = sum_{s<block} Kr[s]V[s]^T
    accumulated in PSUM across s-blocks; O_block = Qr_blk @ A + tril(S_diag,-1) @ V_blk.
"""
import numpy as np

import concourse.bass as bass
import concourse.mybir as mybir
import concourse.tile as tile
from concourse import bacc
from concourse.bass import ts
from concourse.bass_utils import run_bass_kernel_spmd

B, NH, T, N, D = 2, 8, 2048, 256, 256
P = 128
NT = T // P          # 16 t-blocks
HPC = 2              # head-pairs per core
NCORES = 8
F32 = mybir.dt.float32
F32R = mybir.dt.float32r

TRACE = False        # test.py flips this to capture an NTFF profile
LAST_RESULT = None   # BassKernelResults of the most recent run


def _build_nc():
    nc = bacc.Bacc("TRN2", target_bir_lowering=False, debug=False)

    q_d = nc.dram_tensor("q_in", [HPC, T, N], F32, kind="ExternalInput").ap()
    k_d = nc.dram_tensor("k_in", [HPC, T, N], F32, kind="ExternalInput").ap()
    v_d = nc.dram_tensor("v_in", [T, D], F32, kind="ExternalInput").ap()
    cos_d = nc.dram_tensor("cos_in", [T, P], F32, kind="ExternalInput").ap()
    sin_d = nc.dram_tensor("sin_in", [T, P], F32, kind="ExternalInput").ap()
    tril_d = nc.dram_tensor("tril_in", [P, P], F32, kind="ExternalInput").ap()
    ident_d = nc.dram_tensor("ident_in", [P, P], F32, kind="ExternalInput").ap()
    s_d = nc.dram_tensor("s_out", [HPC, T, T], F32, kind="ExternalOutput").ap()
    o_d = nc.dram_tensor("o_out", [HPC, T, D], F32, kind="ExternalOutput").ap()

    with tile.TileContext(nc) as tc:
        with (
            tc.tile_pool(name="const", bufs=1) as constp,
            tc.tile_pool(name="stage", bufs=2) as stagep,
            tc.tile_pool(name="rope", bufs=2) as ropep,
            tc.tile_pool(name="tmp", bufs=1) as tmpp,
            tc.tile_pool(name="tposed", bufs=2) as tpp,
            tc.tile_pool(name="band", bufs=2) as bandp,
            tc.tile_pool(name="small", bufs=2) as smallp,
            tc.tile_pool(name="psA", bufs=1, space="PSUM") as psA,
            tc.tile_pool(name="psS", bufs=3, space="PSUM") as psS,
            tc.tile_pool(name="psO", bufs=2, space="PSUM") as psO,
            tc.tile_pool(name="psT", bufs=2, space="PSUM") as psT,
        ):
            # ---- constants ----
            cos_sb = constp.tile([P, NT, P], F32)
            sin_sb = constp.tile([P, NT, P], F32)
            tril_sb = constp.tile([P, P], F32)
            nc.sync.dma_start(cos_sb[:], cos_d.rearrange("(i p) k -> p i k", p=P))
            nc.sync.dma_start(sin_sb[:], sin_d.rearrange("(i p) k -> p i k", p=P))
            nc.sync.dma_start(tril_sb[:], tril_d)
            ident_sb = constp.tile([P, P], F32)
            nc.sync.dma_start(ident_sb[:], ident_d)
            ident_r = constp.tile([P, P], F32R)
            nc.any.tensor_copy(ident_r[:], ident_sb[:])

            v_raw = stagep.tile([P, NT, D], F32, tag="stage")
            nc.sync.dma_start(v_raw[:], v_d.rearrange("(j p) d -> p j d", p=P))
            v_sb = constp.tile([P, NT, D], F32R)
            nc.any.tensor_copy(v_sb[:], v_raw[:])  # round to f32r

            for bh in range(HPC):
                # ---- phase 0: load + rope (deinterleaved) ----
                def rope(src):
                    raw = stagep.tile([P, NT, N], F32, tag="stage")
                    nc.sync.dma_start(raw[:], src.rearrange("(i p) n -> p i n", p=P))
                    pairs = raw.rearrange("p i (k two) -> p i k two", two=2)
                    xe, xo = pairs[:, :, :, 0], pairs[:, :, :, 1]
                    rr = ropep.tile([P, NT, N], F32R, tag="rope")
                    t0 = tmpp.tile([P, NT, P], F32, tag="ta")
                    t1 = tmpp.tile([P, NT, P], F32, tag="tb")
                    nc.any.tensor_mul(t0[:], xe, cos_sb[:])
                    nc.any.tensor_mul(t1[:], xo, sin_sb[:])
                    nc.any.tensor_sub(rr[:, :, 0:P], t0[:], t1[:])
                    t2 = tmpp.tile([P, NT, P], F32, tag="ta")
                    t3 = tmpp.tile([P, NT, P], F32, tag="tb")
                    nc.any.tensor_mul(t2[:], xo, cos_sb[:])
                    nc.any.tensor_mul(t3[:], xe, sin_sb[:])
                    nc.any.tensor_add(rr[:, :, P:N], t2[:], t3[:])
                    return rr

                qr = rope(q_d[bh])
                kr = rope(k_d[bh])

                # ---- phase 0b: PE-transpose to [n, t] ----
                def transpose_to(rr):
                    rt = tpp.tile([P, 2, T], F32R, tag="tposed")
                    for h in range(2):
                        for j in range(NT // 4):
                            tp = psT.tile([P, 512], F32R, tag="pst")
                            for q in range(4):
                                i = 4 * j + q
                                nc.tensor.transpose(
                                    tp[:, ts(q, P)], rr[:, i, h * P:(h + 1) * P],
                                    ident_r[:],
                                )
                            nc.any.tensor_copy(rt[:, h, ts(j, 512)], tp[:])
                    return rt

                qrt = transpose_to(qr)
                krt = transpose_to(kr)

                # ---- phase 1: main loop over t-blocks ----
                a_ps = psA.tile([P, 2 * D], F32)  # prefix state, persistent bank
                a_sb_prev = None
                for i in range(NT):
                    cw = (i + 1) * P  # causal width
                    band = bandp.tile([P, T], F32, tag="band")
                    for c0 in range(0, cw, 512):
                        w = min(512, cw - c0)
                        s_ps = psS.tile([P, 512], F32, tag="s")
                        nc.tensor.matmul(s_ps[:, :w], qrt[:, 0, ts(i, P)],
                                         krt[:, 0, c0:c0 + w], start=True, stop=False)
                        nc.tensor.matmul(s_ps[:, :w], qrt[:, 1, ts(i, P)],
                                         krt[:, 1, c0:c0 + w], start=False, stop=True)
                        if c0 + w == cw:  # chunk containing the diagonal block
                            if w > P:
                                nc.any.tensor_copy(band[:, c0:cw - P], s_ps[:, :w - P])
                            nc.any.tensor_mul(band[:, cw - P:cw],
                                              s_ps[:, w - P:w], tril_sb[:])
                        else:
                            nc.any.tensor_copy(band[:, c0:c0 + w], s_ps[:, :w])
                    nc.sync.dma_start(s_d[bh, ts(i, P), 0:cw], band[:, 0:cw])

                    # transposed masked diagonal block, for the intra-block O term
                    tsi = psT.tile([P, 512], F32, tag="pst")
                    nc.tensor.transpose(tsi[:, 0:P], band[:, cw - P:cw], ident_sb[:])
                    siit = smallp.tile([P, P], F32R, tag="siit")
                    nc.any.tensor_copy(siit[:], tsi[:, 0:P])

                    # O_i = Qr_i @ A_{<i} + tril(S_ii,-1) @ V_i
                    o_ps = psO.tile([P, D], F32, tag="o")
                    if i > 0:
                        nc.tensor.matmul(o_ps[:], qrt[:, 0, ts(i, P)],
                                         a_sb_prev[:, 0:D], start=True, stop=False,
                                         skip_group_check=True)
                        nc.tensor.matmul(o_ps[:], qrt[:, 1, ts(i, P)],
                                         a_sb_prev[:, D:2 * D], start=False, stop=False,
                                         skip_group_check=True)
                    nc.tensor.matmul(o_ps[:], siit[:], v_sb[:, i, :],
                                     start=(i == 0), stop=True, skip_group_check=True)
                    o_sb = smallp.tile([P, D], F32, tag="osb")
                    nc.any.tensor_copy(o_sb[:], o_ps[:])
                    nc.sync.dma_start(o_d[bh, ts(i, P), :], o_sb[:])

                    # prefix-state update A += Kr_i^T V_i (skip last, never used)
                    if i < NT - 1:
                        nc.tensor.matmul(a_ps[:, 0:D], kr[:, i, 0:P], v_sb[:, i, :],
                                         start=(i == 0), stop=False,
                                         skip_group_check=True)
                        nc.tensor.matmul(a_ps[:, D:2 * D], kr[:, i, P:N], v_sb[:, i, :],
                                         start=False, stop=(i == NT - 2),
                                         skip_group_check=True)
                        a_sb = smallp.tile([P, 2 * D], F32R, tag="asb")
                        nc.any.tensor_copy(a_sb[:], a_ps[:])
                        a_sb_prev = a_sb

    nc.finalize()
    return nc


_NC = None


def _get_nc():
    global _NC
    if _NC is None:
        _NC = _build_nc()
    return _NC


def _tables():
    d = N
    freqs = (1.0 / (10000.0 ** (np.arange(0, d, 2, dtype=np.float32)
                                / np.float32(d)))).astype(np.float32)
    t = np.arange(T, dtype=np.float32)
    ang = (t[:, None] * freqs[None, :]).astype(np.float32)
    return np.cos(ang).astype(np.float32), np.sin(ang).astype(np.float32)


def kernel(Q, K, V):
    global LAST_RESULT
    Q = np.asarray(Q, dtype=np.float32)
    K = np.asarray(K, dtype=np.float32)
    V = np.asarray(V, dtype=np.float32)
    cos, sin = _tables()
    tril = np.tril(np.ones((P, P), dtype=np.float32), -1)

    in_maps = []
    for c in range(NCORES):
        b, h0 = c // 4, 2 * (c % 4)
        in_maps.append({
            "q_in": np.ascontiguousarray(Q[b, h0:h0 + HPC]),
            "k_in": np.ascontiguousarray(K[b, h0:h0 + HPC]),
            "v_in": np.ascontiguousarray(V[b, 0]),
            "cos_in": cos, "sin_in": sin, "tril_in": tril,
            "ident_in": np.eye(P, dtype=np.float32),
        })

    nc = _get_nc()
    res = run_bass_kernel_spmd(nc, in_maps, core_ids=list(range(NCORES)),
                               trace=TRACE)
    LAST_RESULT = res

    out = np.empty((B, NH, T, D), dtype=np.float32)
    scores = np.empty((B, NH, T, T), dtype=np.float32)
    for c in range(NCORES):
        b, h0 = c // 4, 2 * (c % 4)
        out[b, h0:h0 + HPC] = res.results[c]["o_out"]
        scores[b, h0:h0 + HPC] = res.results[c]["s_out"]
    return out, scores


# revision 16
# speedup vs baseline: 1.3698x; 1.3698x over previous
"""Trainium2 Bass kernel: LinearCausalAttention (RoPE + strictly-causal QK^T + scores@V).

Inputs (full): Q (2,8,2048,256) f32, K (2,8,2048,256) f32, V (2,1,2048,256) f32.
Returns (out, scores): out (2,8,2048,256), scores (2,8,2048,2048), both f32.

Sharding: 16 (b,h) pairs over 8 cores, 2 pairs/core; both pairs on a core share b,
so V is loaded once per core. Host-side sharding also pre-permutes layouts
(pure index permutations, no arithmetic) so every DMA is contiguous per
partition:
  - Q is delivered as transposed even/odd feature planes [2, 2, 128, T]; RoPE
    runs directly in the transposed domain and its outputs ARE the QK^T
    stationary operands (no PE transposes for Q at all).
  - K/V/cos/sin are delivered t-block-partition-major.

Per (b,h) on-device pipeline:
  - RoPE with *deinterleaved* feature order (a permutation shared by Q and K
    leaves Q.K^T and scores@V unchanged).
  - K roped in natural layout (kept for the prefix-state update), then
    PE-transposed to [n, t] for the QK^T matmuls (fp32r, full rate).
  - scores: per 128-row t-block, matmul s-chunks (<=512), mask the diagonal
    block with a strictly-lower mask, DMA per chunk; the strictly-upper zero
    region is never written (output buffers are pre-zeroed by the runtime
    contract in both the native and PJRT paths).
  - out via linear-attention prefix state A[n,d] accumulated in PSUM:
    O_blk = Qr_blk @ A_{<blk} + tril(S_diag,-1) @ V_blk, with the masked
    transposed diagonal recomputed as Kr_i @ Qr_i^T to stay off the band path.
"""
import numpy as np

import concourse.bass as bass
import concourse.mybir as mybir
import concourse.tile as tile
from concourse import bacc
from concourse.bass import ts
from concourse.bass_utils import run_bass_kernel_spmd

B, NH, T, N, D = 2, 8, 2048, 256, 256
P = 128
NT = T // P          # 16 t-blocks
HPC = 2              # head-pairs per core
NCORES = 8
F32 = mybir.dt.float32
F32R = mybir.dt.float32r

TRACE = False
LAST_RESULT = None

G = 4                # t-blocks per K pipeline group
NG = NT // G
QC = 1024            # Q transposed-rope chunk (columns of T)


def _build_nc():
    nc = bacc.Bacc("TRN2", target_bir_lowering=False, debug=False)

    q_d = nc.dram_tensor("q_in", [HPC, 2, P, T], F32, kind="ExternalInput").ap()
    k_d = nc.dram_tensor("k_in", [HPC, P, NT, N], F32, kind="ExternalInput").ap()
    v_d = nc.dram_tensor("v_in", [P, NT, D], F32, kind="ExternalInput").ap()
    cst_d = nc.dram_tensor("cst_in", [2, P, T], F32, kind="ExternalInput").ap()
    msk_d = nc.dram_tensor("msk_in", [3, P, P], F32, kind="ExternalInput").ap()
    s_d = nc.dram_tensor("s_out", [HPC, T, T], F32, kind="ExternalOutput").ap()
    o_d = nc.dram_tensor("o_out", [HPC, T, D], F32, kind="ExternalOutput").ap()

    with tile.TileContext(nc) as tc:
        with (
            tc.tile_pool(name="const", bufs=1) as constp,
            tc.tile_pool(name="qstage", bufs=2) as qstagep,
            tc.tile_pool(name="qtmp", bufs=2) as qtmpp,
            tc.tile_pool(name="kstage", bufs=3) as kstagep,
            tc.tile_pool(name="vstage", bufs=1) as vstagep,
            tc.tile_pool(name="ktmp", bufs=2) as ktmpp,
            tc.tile_pool(name="rope", bufs=2) as ropep,
            tc.tile_pool(name="tposed", bufs=2) as tpp,
            tc.tile_pool(name="band", bufs=5) as bandp,
            tc.tile_pool(name="small", bufs=2) as smallp,
            tc.tile_pool(name="psA", bufs=1, space="PSUM") as psA,
            tc.tile_pool(name="psS", bufs=3, space="PSUM") as psS,
            tc.tile_pool(name="psO", bufs=1, space="PSUM") as psO,
        ):
            # ---- constant tiles (DMAs emitted lazily at first use so the
            #      SP issue order matches the critical path) ----
            cst_sb = constp.tile([P, 2, T], F32)
            cos_sb = constp.tile([P, NT, P], F32)
            sin_sb = constp.tile([P, NT, P], F32)
            msk_sb = constp.tile([P, 3, P], F32)
            ident_r = constp.tile([P, P], F32R)
            v_sb = constp.tile([P, NT, D], F32R)
            v_loaded = False
            # one small mask DMA first: ident_r gates the K transposes
            nc.sync.dma_start(msk_sb[:], msk_d.rearrange("m p k -> p m k"))
            tril_sb = msk_sb[:, 0, :]
            triu_sb = msk_sb[:, 1, :]
            ident_sb = msk_sb[:, 2, :]
            nc.any.tensor_copy(ident_r[:], ident_sb)

            def _load_q_chunk(bh, c, ci):
                if bh == 0:
                    nc.sync.dma_start(cst_sb[:, :, c:c + QC],
                                      cst_d[:, :, c:c + QC].rearrange(
                                          "e p t -> p e t"))
                qq = qstagep.tile([P, 2, QC], F32, tag="qq")
                nc.sync.dma_start(qq[:], q_d[bh, :, :, c:c + QC].rearrange(
                    "e p t -> p e t"))
                if bh == 0:
                    # natural-layout rope tables generated on device by
                    # PE-transposing the transposed tables (saves 2MB of
                    # startup DMA traffic)
                    ngq = T // QC
                    for g in range(ci * (NG // ngq), (ci + 1) * (NG // ngq)):
                        gs = slice(g * G, (g + 1) * G)
                        for tbl, dst in ((0, cos_sb), (1, sin_sb)):
                            tp = psS.tile([P, 512], F32, tag="s")
                            for q in range(G):
                                i = g * G + q
                                nc.tensor.transpose(
                                    tp[:, ts(q, P)],
                                    cst_sb[:, tbl, ts(i, P)], ident_sb)
                            nc.scalar.copy(
                                dst[:, gs].rearrange("p g k -> p (g k)"),
                                tp[:])
                return qq

            def _load_k_group(bh, g):
                raw = kstagep.tile([P, G, N], F32, tag="kstage")
                nc.sync.dma_start(raw[:], k_d[bh, :, g * G:(g + 1) * G])
                return raw

            def load_q(bh):
                return [_load_q_chunk(bh, c, ci)
                        for ci, c in enumerate(range(0, T, QC))]

            def load_k(bh):
                return [_load_k_group(bh, g) for g in range(NG)]

            def rope_q(qtiles, bh):
                qrt = tpp.tile([P, 2, T], F32R, tag="tposed")
                for ci, qq in enumerate(qtiles):
                    c = ci * QC
                    qe, qo = qq[:, 0, :], qq[:, 1, :]
                    cc, sc = cst_sb[:, 0, c:c + QC], cst_sb[:, 1, c:c + QC]
                    t0 = qtmpp.tile([P, QC], F32, tag="qta")
                    t1 = qtmpp.tile([P, QC], F32, tag="qtb")
                    nc.any.tensor_mul(t0[:], qe, cc)
                    nc.any.tensor_mul(t1[:], qo, sc)
                    nc.any.tensor_sub(qrt[:, 0, c:c + QC], t0[:], t1[:])
                    t2 = qtmpp.tile([P, QC], F32, tag="qta")
                    t3 = qtmpp.tile([P, QC], F32, tag="qtb")
                    nc.any.tensor_mul(t2[:], qo, cc)
                    nc.gpsimd.tensor_mul(t3[:], qe, sc)
                    nc.any.tensor_add(qrt[:, 1, c:c + QC], t2[:], t3[:])
                return qrt

            def rope_k(ktiles):
                kr = ropep.tile([P, NT, N], F32R, tag="rope")
                krt = tpp.tile([P, 2, T], F32R, tag="tposed")
                for g, raw in enumerate(ktiles):
                    gs = slice(g * G, (g + 1) * G)
                    pairs = raw.rearrange("p i (k two) -> p i k two", two=2)
                    xe, xo = pairs[:, :, :, 0], pairs[:, :, :, 1]
                    cosg, sing = cos_sb[:, gs], sin_sb[:, gs]
                    t0 = ktmpp.tile([P, G, P], F32, tag="kta")
                    t1 = ktmpp.tile([P, G, P], F32, tag="ktb")
                    nc.any.tensor_mul(t0[:], xe, cosg)
                    nc.gpsimd.tensor_mul(t1[:], xo, sing)
                    nc.any.tensor_sub(kr[:, gs, 0:P], t0[:], t1[:])
                    t2 = ktmpp.tile([P, G, P], F32, tag="kta")
                    t3 = ktmpp.tile([P, G, P], F32, tag="ktb")
                    nc.any.tensor_mul(t2[:], xo, cosg)
                    nc.any.tensor_mul(t3[:], xe, sing)
                    nc.any.tensor_add(kr[:, gs, P:N], t2[:], t3[:])
                    for h in range(2):
                        tp = psS.tile([P, 512], F32R, tag="s")
                        for q in range(G):
                            nc.tensor.transpose(
                                tp[:, ts(q, P)],
                                kr[:, g * G + q, h * P:(h + 1) * P],
                                ident_r[:],
                            )
                        nc.scalar.copy(krt[:, h, ts(g, 512)], tp[:])
                return kr, krt

            SBW = 1024  # score-chunk width (2 PSUM banks)
            # small/large interleave: cheap bands (few krt groups needed)
            # alternate with big ones so score writes drain early
            si_order = []
            lo, hi = 0, NT - 1
            while lo <= hi:
                si_order.append(lo)
                if hi != lo:
                    si_order.append(hi)
                lo, hi = lo + 1, hi - 1

            qtiles, ktiles = {}, {}

            def load_qk_interleaved(bh):
                qt, kt = [], []
                qcs = list(range(0, T, QC))
                # q chunk 0 first (starts DVE earliest), then K g0 (unblocks
                # the K rope->transpose->scores chain), then the rest
                qt.append(None); kt.append(None)
                qt[0] = _load_q_chunk(bh, qcs[0], 0)
                kt[0] = _load_k_group(bh, 0)
                for ci in range(1, len(qcs)):
                    qt.append(_load_q_chunk(bh, qcs[ci], ci))
                for g in range(1, NG):
                    kt.append(_load_k_group(bh, g))
                return qt, kt

            qtiles[0], ktiles[0] = load_qk_interleaved(0)
            for bh in range(HPC):
                qrt = rope_q(qtiles[bh], bh)
                kr, krt = rope_k(ktiles[bh])

                if bh == 0:
                    vhalf = NT // 2
                    for vh in range(2):
                        v_raw = vstagep.tile([P, vhalf, D], F32, tag="vstage")
                        vs = slice(vh * vhalf, (vh + 1) * vhalf)
                        nc.sync.dma_start(v_raw[:], v_d[:, vs])
                        nc.scalar.copy(v_sb[:, vs], v_raw[:])
                    # prefetch next head-pair's inputs ahead of the band DMAs
                    qtiles[1] = load_q(1)
                    ktiles[1] = load_k(1)

                # ---- phase 1 ----
                a_ps = psA.tile([P, 2 * D], F32)  # prefix state, persistent bank
                a_sb_prev = None
                o2_sb = None
                for step in range(NT):
                    # scores band si (order decoupled from the O/A recurrence)
                    si = si_order[step]
                    cw = (si + 1) * P
                    for c0 in range(0, cw, SBW):
                        w = min(SBW, cw - c0)
                        s_ps = psS.tile([P, SBW], F32, tag="s")
                        for s0 in range(0, w, 512):
                            sw = min(512, w - s0)
                            nc.tensor.matmul(s_ps[:, s0:s0 + sw],
                                             qrt[:, 0, ts(si, P)],
                                             krt[:, 0, c0 + s0:c0 + s0 + sw],
                                             start=True, stop=False)
                            nc.tensor.matmul(s_ps[:, s0:s0 + sw],
                                             qrt[:, 1, ts(si, P)],
                                             krt[:, 1, c0 + s0:c0 + s0 + sw],
                                             start=False, stop=True)
                        bchunk = bandp.tile([P, SBW], F32, tag="band")
                        if c0 + w == cw:  # chunk containing the diagonal block
                            if w > P:
                                nc.any.tensor_copy(bchunk[:, :w - P], s_ps[:, :w - P])
                            nc.any.tensor_mul(bchunk[:, w - P:w],
                                              s_ps[:, w - P:w], tril_sb)
                        else:
                            nc.any.tensor_copy(bchunk[:, :w], s_ps[:, :w])
                        nc.sync.dma_start(s_d[bh, ts(si, P), c0:c0 + w], bchunk[:, :w])

                    # ---- O/A recurrence (ascending i) ----
                    i = step
                    tsi = psS.tile([P, 512], F32, tag="s")
                    nc.tensor.matmul(tsi[:, 0:P], krt[:, 0, ts(i, P)],
                                     qrt[:, 0, ts(i, P)], start=True, stop=False)
                    nc.tensor.matmul(tsi[:, 0:P], krt[:, 1, ts(i, P)],
                                     qrt[:, 1, ts(i, P)], start=False, stop=True)
                    siit = smallp.tile([P, P], F32R, tag="siit")
                    nc.any.tensor_mul(siit[:], tsi[:, 0:P], triu_sb)

                    o_ps = psO.tile([P, D], F32, tag="o")
                    if i > 0:
                        nc.tensor.matmul(o_ps[:], qrt[:, 0, ts(i, P)],
                                         a_sb_prev[:, 0:D], start=True, stop=False,
                                         skip_group_check=True)
                        nc.tensor.matmul(o_ps[:], qrt[:, 1, ts(i, P)],
                                         a_sb_prev[:, D:2 * D], start=False, stop=False,
                                         skip_group_check=True)
                    nc.tensor.matmul(o_ps[:], siit[:], v_sb[:, i, :],
                                     start=(i == 0), stop=True, skip_group_check=True)
                    # O written in pairs of t-blocks to halve DMA count
                    if i % 2 == 0:
                        o2_sb = smallp.tile([P, 2, D], F32, tag="osb")
                    nc.any.tensor_copy(o2_sb[:, i % 2, :], o_ps[:])
                    if i % 2 == 1:
                        o_dst = o_d[bh].rearrange("(j p) d -> p j d", p=P)[:, i - 1:i + 1, :]
                        nc.sync.dma_start(o_dst, o2_sb[:])

                    # prefix-state update A += Kr_i^T V_i (skip last, never used)
                    if i < NT - 1:
                        nc.tensor.matmul(a_ps[:, 0:D], kr[:, i, 0:P], v_sb[:, i, :],
                                         start=(i == 0), stop=False,
                                         skip_group_check=True)
                        nc.tensor.matmul(a_ps[:, D:2 * D], kr[:, i, P:N], v_sb[:, i, :],
                                         start=False, stop=(i == NT - 2),
                                         skip_group_check=True)
                        a_sb = smallp.tile([P, 2 * D], F32R, tag="asb")
                        nc.any.tensor_copy(a_sb[:], a_ps[:])
                        a_sb_prev = a_sb

    nc.finalize()
    return nc


_NC = None


def _get_nc():
    global _NC
    if _NC is None:
        _NC = _build_nc()
    return _NC


def _tables():
    d = N
    freqs = (1.0 / (10000.0 ** (np.arange(0, d, 2, dtype=np.float32)
                                / np.float32(d)))).astype(np.float32)
    t = np.arange(T, dtype=np.float32)
    ang = (t[:, None] * freqs[None, :]).astype(np.float32)
    return np.cos(ang).astype(np.float32), np.sin(ang).astype(np.float32)


def _blockmajor(x):
    """[T, W] -> [P, NT, W] with t = i*128 + p  (pure permutation)."""
    W = x.shape[-1]
    return np.ascontiguousarray(x.reshape(NT, P, W).transpose(1, 0, 2))


def kernel(Q, K, V):
    global LAST_RESULT
    Q = np.asarray(Q, dtype=np.float32)
    K = np.asarray(K, dtype=np.float32)
    V = np.asarray(V, dtype=np.float32)
    cos, sin = _tables()                      # [T, 128]
    tril = np.tril(np.ones((P, P), dtype=np.float32), -1)
    cst = np.ascontiguousarray(np.stack([cos.T, sin.T]))        # [2, 128, T]
    msk = np.ascontiguousarray(np.stack([tril, tril.T, np.eye(P, dtype=np.float32)]))

    in_maps = []
    for c in range(NCORES):
        b, h0 = c // 4, 2 * (c % 4)
        Qsh = Q[b, h0:h0 + HPC]               # [2, T, 256]
        # even/odd feature planes, transposed: [2(bh), 2(e/o), 128, T]
        q_in = np.ascontiguousarray(
            np.stack([Qsh[:, :, 0::2].transpose(0, 2, 1),
                      Qsh[:, :, 1::2].transpose(0, 2, 1)], axis=1))
        Ksh = K[b, h0:h0 + HPC]
        k_in = np.ascontiguousarray(
            Ksh.reshape(HPC, NT, P, N).transpose(0, 2, 1, 3))
        in_maps.append({
            "q_in": q_in, "k_in": k_in, "v_in": _blockmajor(V[b, 0]),
            "cst_in": cst, "msk_in": msk,
        })

    nc = _get_nc()
    res = run_bass_kernel_spmd(nc, in_maps, core_ids=list(range(NCORES)),
                               trace=TRACE)
    LAST_RESULT = res

    out = np.empty((B, NH, T, D), dtype=np.float32)
    scores = np.empty((B, NH, T, T), dtype=np.float32)
    for c in range(NCORES):
        b, h0 = c // 4, 2 * (c % 4)
        out[b, h0:h0 + HPC] = res.results[c]["o_out"]
        scores[b, h0:h0 + HPC] = res.results[c]["s_out"]
    return out, scores



# revision 19
# speedup vs baseline: 1.3736x; 1.0028x over previous
"""Trainium2 Bass kernel: LinearCausalAttention (RoPE + strictly-causal QK^T + scores@V).

Inputs (full): Q (2,8,2048,256) f32, K (2,8,2048,256) f32, V (2,1,2048,256) f32.
Returns (out, scores): out (2,8,2048,256), scores (2,8,2048,2048), both f32.

Sharding: 16 (b,h) pairs over 8 cores, 2 pairs/core; both pairs on a core share b,
so V is loaded once per core. Host-side sharding also pre-permutes layouts
(pure index permutations, no arithmetic) so every DMA is contiguous per
partition:
  - Q is delivered as transposed even/odd feature planes [2, 2, 128, T]; RoPE
    runs directly in the transposed domain and its outputs ARE the QK^T
    stationary operands (no PE transposes for Q at all).
  - K/V/cos/sin are delivered t-block-partition-major.

Per (b,h) on-device pipeline:
  - RoPE with *deinterleaved* feature order (a permutation shared by Q and K
    leaves Q.K^T and scores@V unchanged).
  - K roped in natural layout (kept for the prefix-state update), then
    PE-transposed to [n, t] for the QK^T matmuls (fp32r, full rate).
  - scores: per 128-row t-block, matmul s-chunks (<=512), mask the diagonal
    block with a strictly-lower mask, DMA per chunk; the strictly-upper zero
    region is never written (output buffers are pre-zeroed by the runtime
    contract in both the native and PJRT paths).
  - out via linear-attention prefix state A[n,d] accumulated in PSUM:
    O_blk = Qr_blk @ A_{<blk} + tril(S_diag,-1) @ V_blk, with the masked
    transposed diagonal recomputed as Kr_i @ Qr_i^T to stay off the band path.
"""
import numpy as np

import concourse.bass as bass
import concourse.mybir as mybir
import concourse.tile as tile
from concourse import bacc
from concourse.bass import ts
from concourse.bass_utils import run_bass_kernel_spmd

B, NH, T, N, D = 2, 8, 2048, 256, 256
P = 128
NT = T // P          # 16 t-blocks
HPC = 2              # head-pairs per core
NCORES = 8
F32 = mybir.dt.float32
F32R = mybir.dt.float32r

TRACE = False
LAST_RESULT = None

G = 4                # t-blocks per K pipeline group
NG = NT // G
QC = 1024            # Q transposed-rope chunk (columns of T)


def _build_nc():
    nc = bacc.Bacc("TRN2", target_bir_lowering=False, debug=False)

    q_d = nc.dram_tensor("q_in", [HPC, 2, P, T], F32, kind="ExternalInput").ap()
    k_d = nc.dram_tensor("k_in", [HPC, P, NT, N], F32, kind="ExternalInput").ap()
    v_d = nc.dram_tensor("v_in", [P, NT, D], F32, kind="ExternalInput").ap()
    cst_d = nc.dram_tensor("cst_in", [2, P, T], F32, kind="ExternalInput").ap()
    msk_d = nc.dram_tensor("msk_in", [3, P, P], F32, kind="ExternalInput").ap()
    s_d = nc.dram_tensor("s_out", [HPC, T, T], F32, kind="ExternalOutput").ap()
    o_d = nc.dram_tensor("o_out", [HPC, T, D], F32, kind="ExternalOutput").ap()

    with tile.TileContext(nc) as tc:
        with (
            tc.tile_pool(name="const", bufs=1) as constp,
            tc.tile_pool(name="qstage", bufs=2) as qstagep,
            tc.tile_pool(name="qtmp", bufs=2) as qtmpp,
            tc.tile_pool(name="kstage", bufs=3) as kstagep,
            tc.tile_pool(name="vstage", bufs=1) as vstagep,
            tc.tile_pool(name="ktmp", bufs=2) as ktmpp,
            tc.tile_pool(name="rope", bufs=2) as ropep,
            tc.tile_pool(name="tposed", bufs=2) as tpp,
            tc.tile_pool(name="band", bufs=5) as bandp,
            tc.tile_pool(name="small", bufs=2) as smallp,
            tc.tile_pool(name="psA", bufs=1, space="PSUM") as psA,
            tc.tile_pool(name="psS", bufs=3, space="PSUM") as psS,
            tc.tile_pool(name="psO", bufs=1, space="PSUM") as psO,
        ):
            # ---- constant tiles (DMAs emitted lazily at first use so the
            #      SP issue order matches the critical path) ----
            cst_sb = constp.tile([P, 2, T], F32)
            cos_sb = constp.tile([P, NT, P], F32)
            sin_sb = constp.tile([P, NT, P], F32)
            msk_sb = constp.tile([P, 3, P], F32)
            ident_r = constp.tile([P, P], F32R)
            v_sb = constp.tile([P, NT, D], F32R)
            v_loaded = False
            # one small mask DMA first: ident_r gates the K transposes
            nc.sync.dma_start(msk_sb[:], msk_d.rearrange("m p k -> p m k"))
            tril_sb = msk_sb[:, 0, :]
            triu_sb = msk_sb[:, 1, :]
            ident_sb = msk_sb[:, 2, :]
            nc.any.tensor_copy(ident_r[:], ident_sb)

            def _load_q_chunk(bh, c, ci):
                if bh == 0:
                    nc.sync.dma_start(cst_sb[:, :, c:c + QC],
                                      cst_d[:, :, c:c + QC].rearrange(
                                          "e p t -> p e t"))
                qq = qstagep.tile([P, 2, QC], F32, tag="qq")
                nc.sync.dma_start(qq[:], q_d[bh, :, :, c:c + QC].rearrange(
                    "e p t -> p e t"))
                if bh == 0:
                    # natural-layout rope tables generated on device by
                    # PE-transposing the transposed tables (saves 2MB of
                    # startup DMA traffic)
                    ngq = T // QC
                    for g in range(ci * (NG // ngq), (ci + 1) * (NG // ngq)):
                        gs = slice(g * G, (g + 1) * G)
                        for tbl, dst in ((0, cos_sb), (1, sin_sb)):
                            tp = psS.tile([P, 512], F32, tag="s")
                            for q in range(G):
                                i = g * G + q
                                nc.tensor.transpose(
                                    tp[:, ts(q, P)],
                                    cst_sb[:, tbl, ts(i, P)], ident_sb)
                            nc.scalar.copy(
                                dst[:, gs].rearrange("p g k -> p (g k)"),
                                tp[:])
                return qq

            def _load_k_group(bh, g):
                raw = kstagep.tile([P, G, N], F32, tag="kstage")
                nc.sync.dma_start(raw[:], k_d[bh, :, g * G:(g + 1) * G])
                return raw

            def load_q(bh):
                return [_load_q_chunk(bh, c, ci)
                        for ci, c in enumerate(range(0, T, QC))]

            def load_k(bh):
                return [_load_k_group(bh, g) for g in range(NG)]

            def rope_q(qtiles, bh):
                qrt = tpp.tile([P, 2, T], F32R, tag="tposed")
                for ci, qq in enumerate(qtiles):
                    c = ci * QC
                    qe, qo = qq[:, 0, :], qq[:, 1, :]
                    cc, sc = cst_sb[:, 0, c:c + QC], cst_sb[:, 1, c:c + QC]
                    t0 = qtmpp.tile([P, QC], F32, tag="qta")
                    t1 = qtmpp.tile([P, QC], F32, tag="qtb")
                    nc.any.tensor_mul(t0[:], qe, cc)
                    nc.any.tensor_mul(t1[:], qo, sc)
                    nc.any.tensor_sub(qrt[:, 0, c:c + QC], t0[:], t1[:])
                    t2 = qtmpp.tile([P, QC], F32, tag="qta")
                    t3 = qtmpp.tile([P, QC], F32, tag="qtb")
                    nc.any.tensor_mul(t2[:], qo, cc)
                    nc.gpsimd.tensor_mul(t3[:], qe, sc)
                    nc.any.tensor_add(qrt[:, 1, c:c + QC], t2[:], t3[:])
                return qrt

            def rope_k(ktiles):
                kr = ropep.tile([P, NT, N], F32R, tag="rope")
                krt = tpp.tile([P, 2, T], F32R, tag="tposed")
                for g, raw in enumerate(ktiles):
                    gs = slice(g * G, (g + 1) * G)
                    pairs = raw.rearrange("p i (k two) -> p i k two", two=2)
                    xe, xo = pairs[:, :, :, 0], pairs[:, :, :, 1]
                    cosg, sing = cos_sb[:, gs], sin_sb[:, gs]
                    t0 = ktmpp.tile([P, G, P], F32, tag="kta")
                    t1 = ktmpp.tile([P, G, P], F32, tag="ktb")
                    nc.any.tensor_mul(t0[:], xe, cosg)
                    nc.gpsimd.tensor_mul(t1[:], xo, sing)
                    nc.any.tensor_sub(kr[:, gs, 0:P], t0[:], t1[:])
                    t2 = ktmpp.tile([P, G, P], F32, tag="kta")
                    t3 = ktmpp.tile([P, G, P], F32, tag="ktb")
                    nc.any.tensor_mul(t2[:], xo, cosg)
                    nc.any.tensor_mul(t3[:], xe, sing)
                    nc.any.tensor_add(kr[:, gs, P:N], t2[:], t3[:])
                    for h in range(2):
                        tp = psS.tile([P, 512], F32R, tag="s")
                        for q in range(G):
                            nc.tensor.transpose(
                                tp[:, ts(q, P)],
                                kr[:, g * G + q, h * P:(h + 1) * P],
                                ident_r[:],
                            )
                        nc.scalar.copy(krt[:, h, ts(g, 512)], tp[:])
                return kr, krt

            SBW = 1024  # score-chunk width (2 PSUM banks)
            # small/large interleave: cheap bands (few krt groups needed)
            # alternate with big ones so score writes drain early
            si_order = []
            lo, hi = 0, NT - 1
            while lo <= hi:
                si_order.append(lo)
                if hi != lo:
                    si_order.append(hi)
                lo, hi = lo + 1, hi - 1

            qtiles, ktiles = {}, {}

            def load_qk_interleaved(bh):
                qt, kt = [], []
                qcs = list(range(0, T, QC))
                # q chunk 0 first (starts DVE earliest), then K g0 (unblocks
                # the K rope->transpose->scores chain), then the rest
                qt.append(None); kt.append(None)
                qt[0] = _load_q_chunk(bh, qcs[0], 0)
                kt[0] = _load_k_group(bh, 0)
                for ci in range(1, len(qcs)):
                    qt.append(_load_q_chunk(bh, qcs[ci], ci))
                for g in range(1, NG):
                    kt.append(_load_k_group(bh, g))
                return qt, kt

            qtiles[0], ktiles[0] = load_qk_interleaved(0)
            for bh in range(HPC):
                qrt = rope_q(qtiles[bh], bh)
                kr, krt = rope_k(ktiles[bh])

                if bh == 0:
                    vhalf = NT // 2
                    for vh in range(2):
                        v_raw = vstagep.tile([P, vhalf, D], F32, tag="vstage")
                        vs = slice(vh * vhalf, (vh + 1) * vhalf)
                        nc.sync.dma_start(v_raw[:], v_d[:, vs])
                        nc.scalar.copy(v_sb[:, vs], v_raw[:])
                    # prefetch next head-pair's inputs ahead of the band DMAs
                    qtiles[1] = load_q(1)
                    ktiles[1] = load_k(1)

                # ---- phase 1 ----
                a_ps = psA.tile([P, 2 * D], F32)  # prefix state, persistent bank
                a_sb_prev = None
                o2_sb = None
                for step in range(NT):
                    # scores band si (order decoupled from the O/A recurrence)
                    si = si_order[step]
                    cw = (si + 1) * P
                    for c0 in range(0, cw, SBW):
                        w = min(SBW, cw - c0)
                        s_ps = psS.tile([P, SBW], F32, tag="s")
                        for s0 in range(0, w, 512):
                            sw = min(512, w - s0)
                            nc.tensor.matmul(s_ps[:, s0:s0 + sw],
                                             qrt[:, 0, ts(si, P)],
                                             krt[:, 0, c0 + s0:c0 + s0 + sw],
                                             start=True, stop=False)
                            nc.tensor.matmul(s_ps[:, s0:s0 + sw],
                                             qrt[:, 1, ts(si, P)],
                                             krt[:, 1, c0 + s0:c0 + s0 + sw],
                                             start=False, stop=True)
                        bchunk = bandp.tile([P, SBW], F32, tag="band")
                        if c0 + w == cw:  # chunk containing the diagonal block
                            if w > P:
                                nc.any.tensor_copy(bchunk[:, :w - P], s_ps[:, :w - P])
                            nc.any.tensor_mul(bchunk[:, w - P:w],
                                              s_ps[:, w - P:w], tril_sb)
                        else:
                            nc.any.tensor_copy(bchunk[:, :w], s_ps[:, :w])
                        nc.sync.dma_start(s_d[bh, ts(si, P), c0:c0 + w], bchunk[:, :w])

                    # ---- O/A recurrence (ascending i) ----
                    i = step
                    # N=256 keeps the fp32r matmul at full rate (N=128 runs at
                    # 1/4); only the 128 columns of t-block i are consumed
                    tb = i * P if i < NT - 1 else T - 2 * P
                    off = i * P - tb
                    tsi = psS.tile([P, 512], F32, tag="s")
                    nc.tensor.matmul(tsi[:, 0:2 * P], krt[:, 0, ts(i, P)],
                                     qrt[:, 0, tb:tb + 2 * P], start=True, stop=False)
                    nc.tensor.matmul(tsi[:, 0:2 * P], krt[:, 1, ts(i, P)],
                                     qrt[:, 1, tb:tb + 2 * P], start=False, stop=True)
                    siit = smallp.tile([P, P], F32R, tag="siit")
                    nc.any.tensor_mul(siit[:], tsi[:, off:off + P], triu_sb)

                    o_ps = psO.tile([P, D], F32, tag="o")
                    if i > 0:
                        nc.tensor.matmul(o_ps[:], qrt[:, 0, ts(i, P)],
                                         a_sb_prev[:, 0:D], start=True, stop=False,
                                         skip_group_check=True)
                        nc.tensor.matmul(o_ps[:], qrt[:, 1, ts(i, P)],
                                         a_sb_prev[:, D:2 * D], start=False, stop=False,
                                         skip_group_check=True)
                    nc.tensor.matmul(o_ps[:], siit[:], v_sb[:, i, :],
                                     start=(i == 0), stop=True, skip_group_check=True)
                    # O written in pairs of t-blocks to halve DMA count
                    if i % 2 == 0:
                        o2_sb = smallp.tile([P, 2, D], F32, tag="osb")
                    nc.any.tensor_copy(o2_sb[:, i % 2, :], o_ps[:])
                    if i % 2 == 1:
                        o_dst = o_d[bh].rearrange("(j p) d -> p j d", p=P)[:, i - 1:i + 1, :]
                        nc.sync.dma_start(o_dst, o2_sb[:])

                    # prefix-state update A += Kr_i^T V_i (skip last, never used)
                    if i < NT - 1:
                        nc.tensor.matmul(a_ps[:, 0:D], kr[:, i, 0:P], v_sb[:, i, :],
                                         start=(i == 0), stop=False,
                                         skip_group_check=True)
                        nc.tensor.matmul(a_ps[:, D:2 * D], kr[:, i, P:N], v_sb[:, i, :],
                                         start=False, stop=(i == NT - 2),
                                         skip_group_check=True)
                        a_sb = smallp.tile([P, 2 * D], F32R, tag="asb")
                        nc.any.tensor_copy(a_sb[:], a_ps[:])
                        a_sb_prev = a_sb

    nc.finalize()
    return nc


_NC = None


def _get_nc():
    global _NC
    if _NC is None:
        _NC = _build_nc()
    return _NC


def _tables():
    d = N
    freqs = (1.0 / (10000.0 ** (np.arange(0, d, 2, dtype=np.float32)
                                / np.float32(d)))).astype(np.float32)
    t = np.arange(T, dtype=np.float32)
    ang = (t[:, None] * freqs[None, :]).astype(np.float32)
    return np.cos(ang).astype(np.float32), np.sin(ang).astype(np.float32)


def _blockmajor(x):
    """[T, W] -> [P, NT, W] with t = i*128 + p  (pure permutation)."""
    W = x.shape[-1]
    return np.ascontiguousarray(x.reshape(NT, P, W).transpose(1, 0, 2))


def kernel(Q, K, V):
    global LAST_RESULT
    Q = np.asarray(Q, dtype=np.float32)
    K = np.asarray(K, dtype=np.float32)
    V = np.asarray(V, dtype=np.float32)
    cos, sin = _tables()                      # [T, 128]
    tril = np.tril(np.ones((P, P), dtype=np.float32), -1)
    cst = np.ascontiguousarray(np.stack([cos.T, sin.T]))        # [2, 128, T]
    msk = np.ascontiguousarray(np.stack([tril, tril.T, np.eye(P, dtype=np.float32)]))

    in_maps = []
    for c in range(NCORES):
        b, h0 = c // 4, 2 * (c % 4)
        Qsh = Q[b, h0:h0 + HPC]               # [2, T, 256]
        # even/odd feature planes, transposed: [2(bh), 2(e/o), 128, T]
        q_in = np.ascontiguousarray(
            np.stack([Qsh[:, :, 0::2].transpose(0, 2, 1),
                      Qsh[:, :, 1::2].transpose(0, 2, 1)], axis=1))
        Ksh = K[b, h0:h0 + HPC]
        k_in = np.ascontiguousarray(
            Ksh.reshape(HPC, NT, P, N).transpose(0, 2, 1, 3))
        in_maps.append({
            "q_in": q_in, "k_in": k_in, "v_in": _blockmajor(V[b, 0]),
            "cst_in": cst, "msk_in": msk,
        })

    nc = _get_nc()
    res = run_bass_kernel_spmd(nc, in_maps, core_ids=list(range(NCORES)),
                               trace=TRACE)
    LAST_RESULT = res

    out = np.empty((B, NH, T, D), dtype=np.float32)
    scores = np.empty((B, NH, T, T), dtype=np.float32)
    for c in range(NCORES):
        b, h0 = c // 4, 2 * (c % 4)
        out[b, h0:h0 + HPC] = res.results[c]["o_out"]
        scores[b, h0:h0 + HPC] = res.results[c]["s_out"]
    return out, scores

